# revision 15
# baseline (speedup 1.0000x reference)
"""Trainium2 Bass kernel for nn_Attention_54580444397738 (gnn_message_passing) v10.

Math per batch b (B=8, N=128, H=256, C=16):
  proj         = local @ W_apair                                     [N, H]
  pre[i,j,:]   = proj[i,:] + proj[j,:] + binary[i,j,:] @ W_binary
                 + b_apair + b_binary                                [N, N, H]
  score[i,j]   = sigmoid(relu(pre[i,j,:]) . W_att + b_att)           [N, N]
  glob         = score @ local                                       [N, H]
  local_pair [i,j,:] = local[i,:] + local[j,:]                       (output 1)
  global_pair[i,j,:] = glob[i,:]  + glob[j,:]                        (output 2)

HW-calibrated cost notes (from A/B probes; the rust cost model is ~0.6x
HW uniformly, and wrong in places):
  - PE effective ~1ns/cycle here -> every streamed PSUM column matters.
  - 1MB HBM write ~5.4us per DMA stream -> output writes alternate
    between BOTH HWDGE rings (sync + scalar).
  - SBUF->SBUF partition-fold gathers are CHEAP on HW (~1-2us), despite
    the model pricing them at 25us; DRAM->single-partition loads are
    genuinely slow.  Flat rows for partition_broadcast use gathers.
  - gpsimd (Q7) tensor_tensor is ~8.9us/stage (2.6cyc/elem) -> only a
    few output adds go there; DVE carries the rest.

Final structure: phase 1 = 8 local_pair stages (gpsimd partition
broadcast of flat rows + DVE bf16 add + 1MB sync-ring write each)
interleaved with the attention chunks; then scores -> glob; phase 2 =
8 global_pair stages.  All output adds on DVE ('D'): Q7 tensor_tensor
measured ~3x DVE, PE/ACT compose ~2x -- both rejected on HW probes,
as were ring-alternation during phase 1 (scalar-ring writes
backpressure ACT mid-relu), lp-stage deferral into phase 2, and
accum_out-fused reductions (ACC_MOD/LP_DEFER knobs remain, off).

The attention j-pair takes TWO matmuls (was 3-5 in earlier versions):
  1) identB @ projW-broadcast        (+proj[i,k] to both halves, N=512)
  2) binT[K=50] @ wxproj-slice       (binary term + bias + BOTH
     proj[j]/proj[j+1] row-broadcasts, N=512)
wxproj is a host-built [128, 32*512] bf16 tile (shipped bit-packed in
f32): per 32-row block m it carries W_binary rows (c 0..15), the bias
row (lane 16), and a proj lane (17) that the device scatters proj rows
into per group; lanes 18..31 are zeros, matched by zeroed pad lanes in
the transposed binary tiles.  The binary transposes run up front,
4 groups per PSUM bank, cast to bf16 in 8 wide ACT copies.
"""

import numpy as np

B, N, H, BIN = 8, 128, 256, 16
NCORES = 8
CPAD = 32        # c dim padded 16 -> 32 so transposed blocks land 32-aligned
IG = 4           # j's per binary-transpose group (4 * 32 = 128)
JS = 16          # j's per output stage tile
NSTAGE = N // JS
NG = N // IG     # 32 transpose groups

# output-stage variants: D = gpsimd bcast + DVE add, P = gpsimd bcast +
# gpsimd add, E = PE compose + ACT cast.
LP_PAT = "DDDDDDDD"
GP_PAT = "DDDDDDDD"
# output-write ring per stage: S = nc.sync (SP HWDGE), A = nc.scalar.
# Scalar-ring writes backpressure the ACT engine mid-relu-stream, so
# phase 1 stays on sync; phase 2 (ACT idle) splits across both rings.
LP_RING = "SSSSSSSS"
GP_RING = "SSSSSSSS"
WX_CHUNKS = 1         # wxproj load/scatter chunks (1 = monolithic)
MAT_IN0 = True        # materialize the replicated x[i,h] add-operand once
                      # per phase (dense in0 may enable DVE 2x packing)
ACC_MOD = 0           # every ACC_MOD-th 4j-tile reduces via ACT accum_out
LP_DEFER = 0          # lp stages deferred into phase 2 (DVE slack there)
SKIP_ATTN = False     # probe knob: drop attention/score work (wrong gp)
SKIP_OUT = False      # probe knob: drop output stages (no lp/gp writes)
SKIP_ACT = False      # probe knob: attention matmuls only (no relu/reduce)

_cache = {}


def _body(tc, io, P, reps=1):
    import concourse.bass as bass
    import concourse.mybir as mybir
    from concourse.masks import make_identity
    from contextlib import ExitStack, nullcontext

    nc = tc.nc
    ts = bass.ts
    f32 = mybir.dt.float32
    f32r = mybir.dt.float32r
    bf16 = mybir.dt.bfloat16
    Relu = mybir.ActivationFunctionType.Relu
    Sigmoid = mybir.ActivationFunctionType.Sigmoid
    AX = mybir.AxisListType.X
    ADD = mybir.AluOpType.add

    local_d, binary_d, wap_d, wxproj_d, batt_d, lp_d, gp_d = io

    ctx = ExitStack()
    with ctx:
        persist = ctx.enter_context(tc.tile_pool(name="persist", bufs=1))
        a2p = ctx.enter_context(tc.tile_pool(name="a2p", bufs=3))
        stagep = ctx.enter_context(tc.tile_pool(name="stagep", bufs=3))
        bcastp = ctx.enter_context(tc.tile_pool(name="bcastp", bufs=3))
        flatp = ctx.enter_context(tc.tile_pool(name="flatp", bufs=1))
        xb16p = (ctx.enter_context(tc.tile_pool(name="xb16p", bufs=1))
                 if MAT_IN0 else None)
        prep = ctx.enter_context(tc.tile_pool(name="prep", bufs=3, space="PSUM"))
        tpp = ctx.enter_context(tc.tile_pool(name="tpp", bufs=2, space="PSUM"))

        # timing builds wrap the whole body in a device-side loop
        loop = tc.For_i(0, reps, 1) if reps > 1 else nullcontext()
        ctx.enter_context(loop)

        # ---------------- persistent setup ----------------
        localSb = persist.tile([N, H], f32, tag="localSb")
        nc.scalar.dma_start(out=localSb, in_=local_d)
        xbL = persist.tile([N, H], bf16, tag="xbL")
        nc.vector.tensor_copy(out=xbL, in_=localSb)
        flL = flatp.tile([1, N * H], bf16, tag="flat")
        nc.scalar.dma_start(out=flL, in_=xbL)

        # binary loads CONTIGUOUSLY as [i, (j,c)] on the SP ring; pad c
        # 16->32: lanes 16 AND 17 are ones (bias lane / proj lane of the
        # K=50 merged matmul), lanes 18..31 zero (vs wxproj zero rows).
        binRaw = persist.tile([128, N * BIN], f32, tag="binRaw")
        nc.sync.dma_start(out=binRaw, in_=binary_d.rearrange("i j c -> i (j c)"))
        binp = persist.tile([128, N * CPAD], f32, tag="binp")
        binp3 = binp.rearrange("p (j c) -> p j c", c=CPAD)
        nc.vector.memset(binp3[:, :, 16:CPAD], 0.0)
        nc.vector.memset(binp3[:, :, 16:18], 1.0)
        nc.scalar.copy(
            out=binp3[:, :, 0:BIN],
            in_=binRaw.rearrange("p (j c) -> p j c", c=BIN))

        identity = persist.tile([128, 128], f32, tag="identity")
        make_identity(nc, identity)
        identB = persist.tile([128, 128], bf16, tag="identB")
        nc.scalar.copy(out=identB, in_=identity)
        onesB = persist.tile([1, 128], bf16, tag="onesB")
        nc.vector.memset(onesB, 1.0)

        # f32 loads, converted to f32r by compute-engine copies
        wapF = persist.tile([128, 2 * H], f32, tag="wapF")
        nc.scalar.dma_start(out=wapF[:, 0:H], in_=wap_d[0:128])
        nc.scalar.dma_start(out=wapF[:, H : 2 * H], in_=wap_d[128:256])
        wapR = persist.tile([128, 2 * H], f32r, tag="wapR")
        nc.scalar.copy(out=wapR, in_=wapF)

        # wxproj ships as bit-packed bf16 pairs in f32 -- no cast needed.
        # Loaded in 4 column chunks (with the proj-lane scatters chunked
        # the same way, below) so attention group g only waits for chunk
        # g//8 instead of the whole 4MB.
        wxprojF = persist.tile([128, 16 * 512], f32, tag="wxprojF")
        wxprojB = wxprojF.bitcast(bf16)   # [128, 32*512] bf16 view

        battRow = persist.tile([1, 1], f32, tag="battRow")
        nc.scalar.dma_start(out=battRow, in_=batt_d.unsqueeze(0))
        battCol = persist.tile([128, 1], f32, tag="battCol")
        nc.gpsimd.partition_broadcast(battCol, battRow)

        # localT = local^T (f32r), then projW = local @ W_apair' (f32r)
        localT = persist.tile([128, H], f32r, tag="localT")
        for hb in range(2):
            tp = tpp.tile([128, 4 * 128], f32, tag="tp")
            nc.tensor.transpose(tp[:, 0:128], localSb[:, ts(hb, 128)], identity)
            nc.scalar.copy(out=localT[:, ts(hb, 128)], in_=tp[:, 0:128])
        pp = prep.tile([128, 4 * H], f32, tag="pre")
        nc.tensor.matmul(pp[:, 0:H], lhsT=localT[:, 0:128], rhs=wapR[:, 0:H],
                         start=True, stop=False)
        nc.tensor.matmul(pp[:, 0:H], lhsT=localT[:, 128:256], rhs=wapR[:, H : 2 * H],
                         start=False, stop=True)
        projWb = persist.tile([128, H], bf16, tag="projWb")
        nc.scalar.copy(out=projWb, in_=pp[:, 0:H])

        # chunked wxproj load + proj-lane scatter: lane 32*m+17 of
        # group-column g holds proj[4g+m] at cols (m%2)*256..+256
        wx3 = wxprojB.rearrange("p (g x) -> p g x", x=512)
        pj4 = projWb.rearrange("(g r) k -> g r k", r=4)
        csz = 8192 // WX_CHUNKS
        gsz = 32 // WX_CHUNKS
        for q in range(WX_CHUNKS):
            nc.scalar.dma_start(out=wxprojF[:, ts(q, csz)],
                                in_=wxproj_d[:, ts(q, csz)])
            for m in range(4):
                cols = slice((m % 2) * H, (m % 2) * H + H)
                row = 32 * m + 17
                nc.scalar.dma_start(
                    out=wx3[row : row + 1, gsz * q : gsz * (q + 1), cols],
                    in_=pj4[gsz * q : gsz * (q + 1), m, :])

        # ALL 32 binary transposes up front (keeps the attention matmul
        # stream free of cross-engine waits); 4 groups share one PSUM
        # bank-pair, cast to bf16 in 8 wide ACT copies.
        binTall = persist.tile([128, NG * 128], bf16, tag="binTall")
        for g4 in range(NG // 4):
            tp = tpp.tile([128, 4 * 128], f32, tag="tp")
            for q in range(4):
                nc.tensor.transpose(tp[:, ts(q, 128)],
                                    binp[:, ts(4 * g4 + q, 128)], identity)
            nc.scalar.copy(out=binTall[:, ts(g4, 512)], in_=tp)

        sp = persist.tile([128, N], f32, tag="sp")
        sm = persist.tile([128, N], f32, tag="sm")
        logitsT = persist.tile([128, N], f32, tag="logitsT")

        # ---------------- helpers ----------------
        a2tiles = {}
        projB2 = projWb.unsqueeze(1).broadcast_to([128, 2, H])

        def attn_chunk(c):
            """pre/relu for j in [8c, 8c+8): two 4-j PSUM tiles (2 banks).
            Per j-pair: one identB matmul (+proj[i,k], N=512) and one K=50
            merged matmul (binary + bias + proj[j]/proj[j+1] rows).  One
            relu (FD=1024) per 4j.  Dot-reduces are emitted one stage
            later (attn_reduce) so DVE never stalls fresh tiles."""
            a2 = a2p.tile([128, 8 * H], bf16, tag="a2")
            a2tiles[c] = a2
            for jj in (0, 4):
                j0 = 8 * c + jj
                g = j0 // IG
                t = 2 * c + jj // 4
                pre = prep.tile([128, 4 * H], f32, tag="pre")
                for h2 in range(2):
                    jl = 2 * h2
                    sl = pre[:, 2 * h2 * H : (2 * h2 + 2) * H]
                    nc.tensor.matmul(sl, lhsT=identB, rhs=projB2,
                                     start=True, stop=False)
                    nc.tensor.matmul(
                        sl,
                        lhsT=binTall[32 * jl : 32 * jl + 50,
                                     g * 128 : (g + 1) * 128],
                        rhs=wxprojB[32 * jl : 32 * jl + 50,
                                    g * 512 : g * 512 + 512],
                        start=False, stop=True, tile_position=(32 * jl, 0))
                if SKIP_ACT:
                    continue
                if ACC_MOD and t % ACC_MOD == ACC_MOD - 1:
                    # fused relu + per-(j,sign) sums on ACT: same a2 bytes
                    # written, but the DVE reduce for these 4 j's vanishes
                    for m in range(4):
                        j = j0 + m
                        if P > 0:
                            nc.scalar.activation(
                                out=a2[:, (jj + m) * H : (jj + m) * H + P],
                                in_=pre[:, m * H : m * H + P], func=Relu,
                                accum_out=sp[:, j : j + 1])
                        if P < H:
                            nc.scalar.activation(
                                out=a2[:, (jj + m) * H + P : (jj + m + 1) * H],
                                in_=pre[:, m * H + P : (m + 1) * H], func=Relu,
                                accum_out=sm[:, j : j + 1])
                else:
                    nc.scalar.activation(out=a2[:, jj * H : (jj + 4) * H],
                                         in_=pre, func=Relu)

        def attn_reduce(c):
            a2 = a2tiles.pop(c)
            if SKIP_ACT:
                return
            a3 = a2.rearrange("p (g k) -> p g k", k=H)
            runs, cur = [], None
            for half in range(2):
                t = 2 * c + half
                if ACC_MOD and t % ACC_MOD == ACC_MOD - 1:
                    cur = None
                    continue
                if cur is None:
                    cur = [4 * half, 4 * half + 4]
                    runs.append(cur)
                else:
                    cur[1] = 4 * half + 4
            for ga, gb in runs:
                if P > 0:
                    nc.vector.tensor_reduce(
                        out=sp[:, 8 * c + ga : 8 * c + gb],
                        in_=a3[:, ga:gb, 0:P], axis=AX, op=ADD)
                if P < H:
                    nc.vector.tensor_reduce(
                        out=sm[:, 8 * c + ga : 8 * c + gb],
                        in_=a3[:, ga:gb, P:H], axis=AX, op=ADD)

        def load_flat(xb):
            fl = flatp.tile([1, N * H], bf16, tag="flat")
            nc.scalar.dma_start(out=fl, in_=xb)
            return fl

        def mat16(xb):
            # replicate x[i,:] JS times into a dense tile (one DVE copy) so
            # every stage add reads two dense operands
            t = xb16p.tile([128, JS * H], bf16, tag="xb16")
            nc.vector.tensor_copy(
                out=t.rearrange("p (j h) -> p j h", h=H),
                in_=xb.unsqueeze(1).broadcast_to([128, JS, H]))
            return t

        def out_stage(s, v, ring, xb, xb16, flA, dram_out):
            fl = flA[0:1, s * JS * H : (s + 1) * JS * H]
            stage = stagep.tile([128, JS * H], bf16, tag="stage")
            st3 = stage.rearrange("p (j h) -> p j h", h=H)
            if v in "DP":
                bt = bcastp.tile([128, JS * H], bf16, tag="bt")
                # bitcast bf16 pairs to f32: partition_broadcast cost scales
                # with element count, so this halves the GPSIMD time
                nc.gpsimd.partition_broadcast(bt.bitcast(f32), fl.bitcast(f32))
                eng = nc.vector if v == "D" else nc.gpsimd
                in0 = (xb16 if xb16 is not None
                       else xb.unsqueeze(1).broadcast_to([128, JS, H]))
                if xb16 is not None:
                    in0 = xb16.rearrange("p (j h) -> p j h", h=H)
                eng.tensor_add(out=st3, in0=in0,
                               in1=bt.rearrange("p (j h) -> p j h", h=H))
            else:  # 'E': all-PE compose in PSUM + ACT cast-copy
                xb2 = xb.unsqueeze(1).broadcast_to([128, 2, H])
                for p in range(8):
                    po = tpp.tile([128, 4 * 128], f32, tag="tp")
                    nc.tensor.matmul(po[:, 0:512], lhsT=identB, rhs=xb2,
                                     start=True, stop=False)
                    nc.tensor.matmul(po[:, 0:512], lhsT=onesB,
                                     rhs=fl[0:1, p * 2 * H : (p + 1) * 2 * H],
                                     start=False, stop=True)
                    nc.scalar.copy(out=stage[:, p * 2 * H : (p + 1) * 2 * H],
                                   in_=po[:, 0:512])
            weng = nc.sync if ring == "S" else nc.scalar
            weng.dma_start(out=dram_out[:, ts(s, JS), :], in_=st3)

        # ---------------- phase 1: local_pair + attention ----------------
        xbL16 = mat16(xbL) if (MAT_IN0 and not SKIP_OUT) else None
        n_ph1 = NSTAGE - LP_DEFER
        ph1_slots = [k * NSTAGE // n_ph1 for k in range(n_ph1)] if n_ph1 else []
        for s in range(NSTAGE):
            if not SKIP_OUT and s in ph1_slots:
                si = ph1_slots.index(s)
                out_stage(si, LP_PAT[si], LP_RING[si], xbL, xbL16, flL, lp_d)
            if not SKIP_ATTN:
                attn_chunk(2 * s)
                attn_chunk(2 * s + 1)
                if s > 0:
                    attn_reduce(2 * (s - 1))
                    attn_reduce(2 * (s - 1) + 1)
        if not SKIP_ATTN:
            attn_reduce(2 * (NSTAGE - 1))
            attn_reduce(2 * (NSTAGE - 1) + 1)

        # ---------------- scores -> glob ----------------
        xbG = persist.tile([N, H], bf16, tag="xbG")
        if SKIP_ATTN or SKIP_ACT:
            nc.vector.tensor_copy(out=xbG, in_=localSb)
        else:
            # logits/score are [i-part, j-free]; transpose for the glob MM
            score = persist.tile([128, N], f32, tag="score")
            if P == 0:
                nc.vector.tensor_scalar_mul(out=logitsT, in0=sm, scalar1=-1.0)
            elif P == H:
                nc.vector.tensor_copy(out=logitsT, in_=sp)
            else:
                nc.vector.tensor_sub(out=logitsT, in0=sp, in1=sm)
            nc.scalar.activation(out=score, in_=logitsT, func=Sigmoid,
                                 bias=battCol)
            tsc = tpp.tile([128, 4 * 128], f32, tag="tp")
            nc.tensor.transpose(tsc[:, 0:128], score, identity)
            scoreT = persist.tile([128, N], f32, tag="scoreT")
            nc.scalar.copy(out=scoreT, in_=tsc[:, 0:128])
            pg = prep.tile([128, 4 * H], f32, tag="pre")
            nc.tensor.matmul(pg[:, 0:H], lhsT=scoreT, rhs=localSb,
                             start=True, stop=True)
            nc.scalar.copy(out=xbG, in_=pg[:, 0:H])

        # ---------------- phase 2: deferred local_pair + global_pair ----
        if not SKIP_OUT:
            ring2 = "SASASASASASASASA"
            w = 0
            for s2 in range(NSTAGE - LP_DEFER, NSTAGE):
                out_stage(s2, "D", ring2[w], xbL, xbL16, flL, lp_d)
                w += 1
            flG = load_flat(xbG)
            xbG16 = mat16(xbG) if MAT_IN0 else None
            for s in range(NSTAGE):
                out_stage(s, GP_PAT[s], ring2[w], xbG, xbG16, flG, gp_d)
                w += 1
        else:
            nc.sync.dma_start(out=gp_d[0:1, 0:1, :], in_=xbG[0:1, :])
            nc.sync.dma_start(out=lp_d[0:1, 0:1, :], in_=xbL[0:1, :])


def _build(P, reps=1):
    import concourse.bass as bass  # noqa: F401
    from concourse import bacc
    import concourse.mybir as mybir
    import concourse.tile as tile

    f32 = mybir.dt.float32
    bf16 = mybir.dt.bfloat16
    nc = bacc.Bacc(
        "TRN2",
        target_bir_lowering=False,
        debug=False,
        enable_asserts=False,
        num_devices=NCORES,
    )
    io = (
        nc.dram_tensor("local", [N, H], f32, kind="ExternalInput").ap(),
        nc.dram_tensor("binary", [N, N, BIN], f32, kind="ExternalInput").ap(),
        nc.dram_tensor("w_apair", [H, H], f32, kind="ExternalInput").ap(),
        nc.dram_tensor("wxproj", [128, 16 * 512], f32, kind="ExternalInput").ap(),
        nc.dram_tensor("b_att", [1], f32, kind="ExternalInput").ap(),
        nc.dram_tensor("out_lp", [N, N, H], bf16, kind="ExternalOutput").ap(),
        nc.dram_tensor("out_gp", [N, N, H], bf16, kind="ExternalOutput").ap(),
    )
    with tile.TileContext(nc) as tc:
        _body(tc, io, P, reps=reps)
    nc.compile()
    return nc


def _bf16_bits(x):
    a = np.ascontiguousarray(x, np.float32).view(np.uint32)
    return (a + 0x7FFF + ((a >> 16) & 1)) >> 16


def _pack_bf16(x):
    """[..., 2n] f32 -> [..., n] f32 whose bits are packed bf16 pairs
    (elem0 in the low half-word: SBUF little-endian layout)."""
    b = _bf16_bits(x)
    return ((b[..., 1::2] << 16) | b[..., 0::2]).astype(np.uint32).view(np.float32)


def _prep_inputs(inputs):
    f = lambda x: np.ascontiguousarray(np.asarray(x), dtype=np.float32)
    w_att = f(inputs["W_att"]).reshape(-1)
    perm = np.argsort((w_att <= 0).astype(np.int32), kind="stable")
    P = int((w_att > 0).sum())
    a = np.abs(w_att[perm])
    wap = f(inputs["W_apair"])[:, perm] * a[None, :]
    wbin = f(inputs["W_binary"])[:, perm] * a[None, :]
    bias = (f(inputs["b_apair"]) + f(inputs["b_binary"]))[perm] * a
    # wxproj [128 rows, 32 group-cols, 512]: block m rows 32m..32m+15 =
    # W_binary, row 32m+16 = bias, into cols (m%2)*256..+256 for every
    # group; proj lanes (32m+17) and pad rows stay zero (device fills
    # proj).  Shipped bit-packed as bf16 pairs in f32.
    wxproj = np.zeros((128, 32, 2 * H), np.float32)
    for m in range(4):
        cols = slice((m % 2) * H, (m % 2) * H + H)
        wxproj[32 * m : 32 * m + 16, :, cols] = wbin[:, None, :]
        wxproj[32 * m + 16, :, cols] = bias
    wxprojp = _pack_bf16(wxproj.reshape(128, 32 * 2 * H))
    shared = {
        "w_apair": np.ascontiguousarray(wap),
        "wxproj": wxprojp,
        "b_att": f(inputs["b_att"]),
    }
    local = f(inputs["local_feats"])
    binary = f(inputs["binary_feats"])
    in_maps = [
        {"local": local[c], "binary": binary[c], **shared} for c in range(NCORES)
    ]
    return P, in_maps


def _get_nc(P):
    if P not in _cache:
        _cache[P] = _build(P)
    return _cache[P]


def _run(inputs, trace=False):
    from concourse.bass_utils import run_bass_kernel_spmd

    P, in_maps = _prep_inputs(inputs)
    nc = _get_nc(P)
    res = run_bass_kernel_spmd(
        nc, in_maps, core_ids=list(range(NCORES)), trace=trace
    )
    lp = np.stack([np.asarray(r["out_lp"]).astype(np.float32)
                   for r in res.results])
    gp = np.stack([np.asarray(r["out_gp"]).astype(np.float32)
                   for r in res.results])
    return (lp, gp), res


def kernel(**inputs):
    out, _ = _run(inputs, trace=False)
    return out


# revision 16
# speedup vs baseline: 1.0103x; 1.0103x over previous
"""Trainium2 Bass kernel for nn_Attention_54580444397738 (gnn_message_passing) v10.

Math per batch b (B=8, N=128, H=256, C=16):
  proj         = local @ W_apair                                     [N, H]
  pre[i,j,:]   = proj[i,:] + proj[j,:] + binary[i,j,:] @ W_binary
                 + b_apair + b_binary                                [N, N, H]
  score[i,j]   = sigmoid(relu(pre[i,j,:]) . W_att + b_att)           [N, N]
  glob         = score @ local                                       [N, H]
  local_pair [i,j,:] = local[i,:] + local[j,:]                       (output 1)
  global_pair[i,j,:] = glob[i,:]  + glob[j,:]                        (output 2)

HW-calibrated cost notes (from A/B probes; the rust cost model is ~0.6x
HW uniformly, and wrong in places):
  - PE effective ~1ns/cycle here -> every streamed PSUM column matters.
  - 1MB HBM write ~5.4us per DMA stream -> output writes alternate
    between BOTH HWDGE rings (sync + scalar).
  - SBUF->SBUF partition-fold gathers are CHEAP on HW (~1-2us), despite
    the model pricing them at 25us; DRAM->single-partition loads are
    genuinely slow.  Flat rows for partition_broadcast use gathers.
  - gpsimd (Q7) tensor_tensor is ~8.9us/stage (2.6cyc/elem) -> only a
    few output adds go there; DVE carries the rest.

Final structure: phase 1 = 8 local_pair stages (gpsimd partition
broadcast of flat rows + DVE bf16 add + 1MB sync-ring write each)
interleaved with the attention chunks; then scores -> glob; phase 2 =
8 global_pair stages.  All output adds on DVE ('D'): Q7 tensor_tensor
measured ~3x DVE, PE/ACT compose ~2x -- both rejected on HW probes,
as were ring-alternation during phase 1 (scalar-ring writes
backpressure ACT mid-relu), lp-stage deferral into phase 2, and
accum_out-fused reductions (ACC_MOD/LP_DEFER knobs remain, off).

The attention j-pair takes TWO matmuls (was 3-5 in earlier versions):
  1) identB @ projW-broadcast        (+proj[i,k] to both halves, N=512)
  2) binT[K=50] @ wxproj-slice       (binary term + bias + BOTH
     proj[j]/proj[j+1] row-broadcasts, N=512)
wxproj is a host-built [128, 32*512] bf16 tile (shipped bit-packed in
f32): per 32-row block m it carries W_binary rows (c 0..15), the bias
row (lane 16), and a proj lane (17) that the device scatters proj rows
into per group; lanes 18..31 are zeros, matched by zeroed pad lanes in
the transposed binary tiles.  The binary transposes run up front,
4 groups per PSUM bank, cast to bf16 in 8 wide ACT copies.
"""

import numpy as np

B, N, H, BIN = 8, 128, 256, 16
NCORES = 8
CPAD = 32        # c dim padded 16 -> 32 so transposed blocks land 32-aligned
IG = 4           # j's per binary-transpose group (4 * 32 = 128)
JS = 16          # j's per output stage tile
NSTAGE = N // JS
NG = N // IG     # 32 transpose groups

# output-stage variants: D = gpsimd bcast + DVE add, P = gpsimd bcast +
# gpsimd add, E = PE compose + ACT cast.
LP_PAT = "DDDDDDDD"
GP_PAT = "DDDDDDDD"
# output-write ring per stage: S = nc.sync (SP HWDGE), A = nc.scalar.
# Scalar-ring writes backpressure the ACT engine mid-relu-stream, so
# phase 1 stays on sync; phase 2 (ACT idle) splits across both rings.
LP_RING = "SSSSSSSS"
GP_RING = "SSSSSSSS"
WX_CHUNKS = 1         # wxproj load/scatter chunks (1 = monolithic)
MAT_IN0 = True        # materialize the replicated x[i,h] add-operand once
                      # per phase (dense in0 may enable DVE 2x packing)
ACC_MOD = 0           # every ACC_MOD-th 4j-tile reduces via ACT accum_out
LP_DEFER = 0          # lp stages deferred into phase 2 (DVE slack there)
SKIP_ATTN = False     # probe knob: drop attention/score work (wrong gp)
SKIP_OUT = False      # probe knob: drop output stages (no lp/gp writes)
SKIP_ACT = False      # probe knob: attention matmuls only (no relu/reduce)

_cache = {}


def _body(tc, io, P, reps=1):
    import concourse.bass as bass
    import concourse.mybir as mybir
    from concourse.masks import make_identity
    from contextlib import ExitStack, nullcontext

    nc = tc.nc
    ts = bass.ts
    f32 = mybir.dt.float32
    f32r = mybir.dt.float32r
    bf16 = mybir.dt.bfloat16
    Relu = mybir.ActivationFunctionType.Relu
    Sigmoid = mybir.ActivationFunctionType.Sigmoid
    AX = mybir.AxisListType.X
    ADD = mybir.AluOpType.add

    local_d, binary_d, wap_d, wxproj_d, batt_d, lp_d, gp_d = io

    ctx = ExitStack()
    with ctx:
        persist = ctx.enter_context(tc.tile_pool(name="persist", bufs=1))
        a2p = ctx.enter_context(tc.tile_pool(name="a2p", bufs=3))
        stagep = ctx.enter_context(tc.tile_pool(name="stagep", bufs=3))
        bcastp = ctx.enter_context(tc.tile_pool(name="bcastp", bufs=3))
        flatp = ctx.enter_context(tc.tile_pool(name="flatp", bufs=1))
        xb16p = (ctx.enter_context(tc.tile_pool(name="xb16p", bufs=1))
                 if MAT_IN0 else None)
        prep = ctx.enter_context(tc.tile_pool(name="prep", bufs=3, space="PSUM"))
        tpp = ctx.enter_context(tc.tile_pool(name="tpp", bufs=2, space="PSUM"))

        # timing builds wrap the whole body in a device-side loop
        loop = tc.For_i(0, reps, 1) if reps > 1 else nullcontext()
        ctx.enter_context(loop)

        # ---------------- persistent setup ----------------
        localSb = persist.tile([N, H], f32, tag="localSb")
        nc.scalar.dma_start(out=localSb, in_=local_d)
        xbL = persist.tile([N, H], bf16, tag="xbL")
        nc.vector.tensor_copy(out=xbL, in_=localSb)
        flL = flatp.tile([1, N * H], bf16, tag="flat")
        nc.scalar.dma_start(out=flL, in_=xbL)

        # binary loads CONTIGUOUSLY as [i, (j,c)] on the SP ring; pad c
        # 16->32: lanes 16 AND 17 are ones (bias lane / proj lane of the
        # K=50 merged matmul), lanes 18..31 zero (vs wxproj zero rows).
        binRaw = persist.tile([128, N * BIN], f32, tag="binRaw")
        nc.sync.dma_start(out=binRaw, in_=binary_d.rearrange("i j c -> i (j c)"))
        binp = persist.tile([128, N * CPAD], f32, tag="binp")
        binp3 = binp.rearrange("p (j c) -> p j c", c=CPAD)
        nc.vector.memset(binp3[:, :, 16:CPAD], 0.0)
        nc.vector.memset(binp3[:, :, 16:18], 1.0)
        nc.scalar.copy(
            out=binp3[:, :, 0:BIN],
            in_=binRaw.rearrange("p (j c) -> p j c", c=BIN))

        identity = persist.tile([128, 128], f32, tag="identity")
        make_identity(nc, identity)
        identB = persist.tile([128, 128], bf16, tag="identB")
        nc.scalar.copy(out=identB, in_=identity)
        onesB = persist.tile([1, 128], bf16, tag="onesB")
        nc.vector.memset(onesB, 1.0)

        # f32 loads, converted to f32r by compute-engine copies
        wapF = persist.tile([128, 2 * H], f32, tag="wapF")
        nc.scalar.dma_start(out=wapF[:, 0:H], in_=wap_d[0:128])
        nc.scalar.dma_start(out=wapF[:, H : 2 * H], in_=wap_d[128:256])
        wapR = persist.tile([128, 2 * H], f32r, tag="wapR")
        nc.scalar.copy(out=wapR, in_=wapF)

        # wxproj ships as bit-packed bf16 pairs in f32 -- no cast needed.
        # Loaded in 4 column chunks (with the proj-lane scatters chunked
        # the same way, below) so attention group g only waits for chunk
        # g//8 instead of the whole 4MB.
        wxprojF = persist.tile([128, 16 * 512], f32, tag="wxprojF")
        wxprojB = wxprojF.bitcast(bf16)   # [128, 32*512] bf16 view

        battRow = persist.tile([1, 1], f32, tag="battRow")
        nc.scalar.dma_start(out=battRow, in_=batt_d.unsqueeze(0))
        battCol = persist.tile([128, 1], f32, tag="battCol")
        nc.gpsimd.partition_broadcast(battCol, battRow)

        # localT = local^T (f32r), then projW = local @ W_apair' (f32r)
        localT = persist.tile([128, H], f32r, tag="localT")
        for hb in range(2):
            tp = tpp.tile([128, 4 * 128], f32, tag="tp")
            nc.tensor.transpose(tp[:, 0:128], localSb[:, ts(hb, 128)], identity)
            nc.scalar.copy(out=localT[:, ts(hb, 128)], in_=tp[:, 0:128])
        pp = prep.tile([128, 4 * H], f32, tag="pre")
        nc.tensor.matmul(pp[:, 0:H], lhsT=localT[:, 0:128], rhs=wapR[:, 0:H],
                         start=True, stop=False)
        nc.tensor.matmul(pp[:, 0:H], lhsT=localT[:, 128:256], rhs=wapR[:, H : 2 * H],
                         start=False, stop=True)
        projWb = persist.tile([128, H], bf16, tag="projWb")
        nc.scalar.copy(out=projWb, in_=pp[:, 0:H])

        # chunked wxproj load + proj-lane scatter: lane 32*m+17 of
        # group-column g holds proj[4g+m] at cols (m%2)*256..+256
        wx3 = wxprojB.rearrange("p (g x) -> p g x", x=512)
        pj4 = projWb.rearrange("(g r) k -> g r k", r=4)
        csz = 8192 // WX_CHUNKS
        gsz = 32 // WX_CHUNKS
        for q in range(WX_CHUNKS):
            nc.scalar.dma_start(out=wxprojF[:, ts(q, csz)],
                                in_=wxproj_d[:, ts(q, csz)])
            for m in range(4):
                cols = slice((m % 2) * H, (m % 2) * H + H)
                row = 32 * m + 17
                nc.scalar.dma_start(
                    out=wx3[row : row + 1, gsz * q : gsz * (q + 1), cols],
                    in_=pj4[gsz * q : gsz * (q + 1), m, :])

        # ALL 32 binary transposes up front (keeps the attention matmul
        # stream free of cross-engine waits); 4 groups share one PSUM
        # bank-pair, cast to bf16 in 8 wide ACT copies.
        binTall = persist.tile([128, NG * 128], bf16, tag="binTall")
        for g4 in range(NG // 4):
            tp = tpp.tile([128, 4 * 128], f32, tag="tp")
            for q in range(4):
                nc.tensor.transpose(tp[:, ts(q, 128)],
                                    binp[:, ts(4 * g4 + q, 128)], identity)
            nc.scalar.copy(out=binTall[:, ts(g4, 512)], in_=tp)

        sp = persist.tile([128, N], f32, tag="sp")
        sm = persist.tile([128, N], f32, tag="sm")
        logitsT = persist.tile([128, N], f32, tag="logitsT")

        # ---------------- helpers ----------------
        a2tiles = {}
        projB2 = projWb.unsqueeze(1).broadcast_to([128, 2, H])

        def attn_chunk(c):
            """pre/relu for j in [8c, 8c+8): two 4-j PSUM tiles (2 banks).
            Per j-pair: one identB matmul (+proj[i,k], N=512) and one K=50
            merged matmul (binary + bias + proj[j]/proj[j+1] rows).  One
            relu (FD=1024) per 4j.  Dot-reduces are emitted one stage
            later (attn_reduce) so DVE never stalls fresh tiles."""
            a2 = a2p.tile([128, 8 * H], bf16, tag="a2")
            a2tiles[c] = a2
            for jj in (0, 4):
                j0 = 8 * c + jj
                g = j0 // IG
                t = 2 * c + jj // 4
                pre = prep.tile([128, 4 * H], f32, tag="pre")
                for h2 in range(2):
                    jl = 2 * h2
                    sl = pre[:, 2 * h2 * H : (2 * h2 + 2) * H]
                    nc.tensor.matmul(sl, lhsT=identB, rhs=projB2,
                                     start=True, stop=False)
                    nc.tensor.matmul(
                        sl,
                        lhsT=binTall[32 * jl : 32 * jl + 50,
                                     g * 128 : (g + 1) * 128],
                        rhs=wxprojB[32 * jl : 32 * jl + 50,
                                    g * 512 : g * 512 + 512],
                        start=False, stop=True, tile_position=(32 * jl, 0))
                if SKIP_ACT:
                    continue
                if ACC_MOD and t % ACC_MOD == ACC_MOD - 1:
                    # fused relu + per-(j,sign) sums on ACT: same a2 bytes
                    # written, but the DVE reduce for these 4 j's vanishes
                    for m in range(4):
                        j = j0 + m
                        if P > 0:
                            nc.scalar.activation(
                                out=a2[:, (jj + m) * H : (jj + m) * H + P],
                                in_=pre[:, m * H : m * H + P], func=Relu,
                                accum_out=sp[:, j : j + 1])
                        if P < H:
                            nc.scalar.activation(
                                out=a2[:, (jj + m) * H + P : (jj + m + 1) * H],
                                in_=pre[:, m * H + P : (m + 1) * H], func=Relu,
                                accum_out=sm[:, j : j + 1])
                else:
                    nc.scalar.activation(out=a2[:, jj * H : (jj + 4) * H],
                                         in_=pre, func=Relu)

        def attn_reduce(c):
            a2 = a2tiles.pop(c)
            if SKIP_ACT:
                return
            a3 = a2.rearrange("p (g k) -> p g k", k=H)
            runs, cur = [], None
            for half in range(2):
                t = 2 * c + half
                if ACC_MOD and t % ACC_MOD == ACC_MOD - 1:
                    cur = None
                    continue
                if cur is None:
                    cur = [4 * half, 4 * half + 4]
                    runs.append(cur)
                else:
                    cur[1] = 4 * half + 4
            for ga, gb in runs:
                if P > 0:
                    nc.vector.tensor_reduce(
                        out=sp[:, 8 * c + ga : 8 * c + gb],
                        in_=a3[:, ga:gb, 0:P], axis=AX, op=ADD)
                if P < H:
                    nc.vector.tensor_reduce(
                        out=sm[:, 8 * c + ga : 8 * c + gb],
                        in_=a3[:, ga:gb, P:H], axis=AX, op=ADD)

        def load_flat(xb):
            fl = flatp.tile([1, N * H], bf16, tag="flat")
            nc.scalar.dma_start(out=fl, in_=xb)
            return fl

        def mat16(xb):
            # replicate x[i,:] JS times into a dense tile (one DVE copy) so
            # every stage add reads two dense operands
            t = xb16p.tile([128, JS * H], bf16, tag="xb16")
            nc.vector.tensor_copy(
                out=t.rearrange("p (j h) -> p j h", h=H),
                in_=xb.unsqueeze(1).broadcast_to([128, JS, H]))
            return t

        def out_stage(s, v, ring, xb, xb16, flA, dram_out):
            fl = flA[0:1, s * JS * H : (s + 1) * JS * H]
            stage = stagep.tile([128, JS * H], bf16, tag="stage")
            st3 = stage.rearrange("p (j h) -> p j h", h=H)
            if v in "DP":
                bt = bcastp.tile([128, JS * H], bf16, tag="bt")
                # bitcast bf16 pairs to f32: partition_broadcast cost scales
                # with element count, so this halves the GPSIMD time
                nc.gpsimd.partition_broadcast(bt.bitcast(f32), fl.bitcast(f32))
                eng = nc.vector if v == "D" else nc.gpsimd
                in0 = (xb16 if xb16 is not None
                       else xb.unsqueeze(1).broadcast_to([128, JS, H]))
                if xb16 is not None:
                    in0 = xb16.rearrange("p (j h) -> p j h", h=H)
                eng.tensor_add(out=st3, in0=in0,
                               in1=bt.rearrange("p (j h) -> p j h", h=H))
            else:  # 'E': all-PE compose in PSUM + ACT cast-copy
                xb2 = xb.unsqueeze(1).broadcast_to([128, 2, H])
                for p in range(8):
                    po = tpp.tile([128, 4 * 128], f32, tag="tp")
                    nc.tensor.matmul(po[:, 0:512], lhsT=identB, rhs=xb2,
                                     start=True, stop=False)
                    nc.tensor.matmul(po[:, 0:512], lhsT=onesB,
                                     rhs=fl[0:1, p * 2 * H : (p + 1) * 2 * H],
                                     start=False, stop=True)
                    nc.scalar.copy(out=stage[:, p * 2 * H : (p + 1) * 2 * H],
                                   in_=po[:, 0:512])
            weng = nc.sync if ring == "S" else nc.scalar
            weng.dma_start(out=dram_out[:, ts(s, JS), :], in_=st3)

        # ---------------- phase 1: local_pair + attention ----------------
        xbL16 = mat16(xbL) if (MAT_IN0 and not SKIP_OUT) else None
        n_ph1 = NSTAGE - LP_DEFER
        ph1_slots = [k * NSTAGE // n_ph1 for k in range(n_ph1)] if n_ph1 else []
        for s in range(NSTAGE):
            if not SKIP_OUT and s in ph1_slots:
                si = ph1_slots.index(s)
                out_stage(si, LP_PAT[si], LP_RING[si], xbL, xbL16, flL, lp_d)
            if not SKIP_ATTN:
                attn_chunk(2 * s)
                attn_chunk(2 * s + 1)
                if s > 0:
                    attn_reduce(2 * (s - 1))
                    attn_reduce(2 * (s - 1) + 1)
        if not SKIP_ATTN:
            attn_reduce(2 * (NSTAGE - 1))
            attn_reduce(2 * (NSTAGE - 1) + 1)

        # ---------------- scores -> glob ----------------
        xbG = persist.tile([N, H], bf16, tag="xbG")
        if SKIP_ATTN or SKIP_ACT:
            nc.vector.tensor_copy(out=xbG, in_=localSb)
        else:
            # logits/score are [i-part, j-free]; transpose for the glob MM
            score = persist.tile([128, N], f32, tag="score")
            if P == 0:
                nc.vector.tensor_scalar_mul(out=logitsT, in0=sm, scalar1=-1.0)
            elif P == H:
                nc.vector.tensor_copy(out=logitsT, in_=sp)
            else:
                nc.vector.tensor_sub(out=logitsT, in0=sp, in1=sm)
            nc.scalar.activation(out=score, in_=logitsT, func=Sigmoid,
                                 bias=battCol)
            tsc = tpp.tile([128, 4 * 128], f32, tag="tp")
            nc.tensor.transpose(tsc[:, 0:128], score, identity)
            scoreT = persist.tile([128, N], f32, tag="scoreT")
            nc.scalar.copy(out=scoreT, in_=tsc[:, 0:128])
            pg = prep.tile([128, 4 * H], f32, tag="pre")
            nc.tensor.matmul(pg[:, 0:H], lhsT=scoreT, rhs=localSb,
                             start=True, stop=True)
            nc.scalar.copy(out=xbG, in_=pg[:, 0:H])

        # ---------------- phase 2: deferred local_pair + global_pair ----
        if not SKIP_OUT:
            ring2 = "SASASASASASASASA"
            w = 0
            for s2 in range(NSTAGE - LP_DEFER, NSTAGE):
                out_stage(s2, "D", ring2[w], xbL, xbL16, flL, lp_d)
                w += 1
            flG = load_flat(xbG)
            xbG16 = mat16(xbG) if MAT_IN0 else None
            for s in range(NSTAGE):
                x16 = xbG16 if s >= 2 else None
                out_stage(s, GP_PAT[s], ring2[w], xbG, x16, flG, gp_d)
                w += 1
        else:
            nc.sync.dma_start(out=gp_d[0:1, 0:1, :], in_=xbG[0:1, :])
            nc.sync.dma_start(out=lp_d[0:1, 0:1, :], in_=xbL[0:1, :])


def _build(P, reps=1):
    import concourse.bass as bass  # noqa: F401
    from concourse import bacc
    import concourse.mybir as mybir
    import concourse.tile as tile

    f32 = mybir.dt.float32
    bf16 = mybir.dt.bfloat16
    nc = bacc.Bacc(
        "TRN2",
        target_bir_lowering=False,
        debug=False,
        enable_asserts=False,
        num_devices=NCORES,
    )
    io = (
        nc.dram_tensor("local", [N, H], f32, kind="ExternalInput").ap(),
        nc.dram_tensor("binary", [N, N, BIN], f32, kind="ExternalInput").ap(),
        nc.dram_tensor("w_apair", [H, H], f32, kind="ExternalInput").ap(),
        nc.dram_tensor("wxproj", [128, 16 * 512], f32, kind="ExternalInput").ap(),
        nc.dram_tensor("b_att", [1], f32, kind="ExternalInput").ap(),
        nc.dram_tensor("out_lp", [N, N, H], bf16, kind="ExternalOutput").ap(),
        nc.dram_tensor("out_gp", [N, N, H], bf16, kind="ExternalOutput").ap(),
    )
    with tile.TileContext(nc) as tc:
        _body(tc, io, P, reps=reps)
    nc.compile()
    return nc


def _bf16_bits(x):
    a = np.ascontiguousarray(x, np.float32).view(np.uint32)
    return (a + 0x7FFF + ((a >> 16) & 1)) >> 16


def _pack_bf16(x):
    """[..., 2n] f32 -> [..., n] f32 whose bits are packed bf16 pairs
    (elem0 in the low half-word: SBUF little-endian layout)."""
    b = _bf16_bits(x)
    return ((b[..., 1::2] << 16) | b[..., 0::2]).astype(np.uint32).view(np.float32)


def _prep_inputs(inputs):
    f = lambda x: np.ascontiguousarray(np.asarray(x), dtype=np.float32)
    w_att = f(inputs["W_att"]).reshape(-1)
    perm = np.argsort((w_att <= 0).astype(np.int32), kind="stable")
    P = int((w_att > 0).sum())
    a = np.abs(w_att[perm])
    wap = f(inputs["W_apair"])[:, perm] * a[None, :]
    wbin = f(inputs["W_binary"])[:, perm] * a[None, :]
    bias = (f(inputs["b_apair"]) + f(inputs["b_binary"]))[perm] * a
    # wxproj [128 rows, 32 group-cols, 512]: block m rows 32m..32m+15 =
    # W_binary, row 32m+16 = bias, into cols (m%2)*256..+256 for every
    # group; proj lanes (32m+17) and pad rows stay zero (device fills
    # proj).  Shipped bit-packed as bf16 pairs in f32.
    wxproj = np.zeros((128, 32, 2 * H), np.float32)
    for m in range(4):
        cols = slice((m % 2) * H, (m % 2) * H + H)
        wxproj[32 * m : 32 * m + 16, :, cols] = wbin[:, None, :]
        wxproj[32 * m + 16, :, cols] = bias
    wxprojp = _pack_bf16(wxproj.reshape(128, 32 * 2 * H))
    shared = {
        "w_apair": np.ascontiguousarray(wap),
        "wxproj": wxprojp,
        "b_att": f(inputs["b_att"]),
    }
    local = f(inputs["local_feats"])
    binary = f(inputs["binary_feats"])
    in_maps = [
        {"local": local[c], "binary": binary[c], **shared} for c in range(NCORES)
    ]
    return P, in_maps


def _get_nc(P):
    if P not in _cache:
        _cache[P] = _build(P)
    return _cache[P]


def _run(inputs, trace=False):
    from concourse.bass_utils import run_bass_kernel_spmd

    P, in_maps = _prep_inputs(inputs)
    nc = _get_nc(P)
    res = run_bass_kernel_spmd(
        nc, in_maps, core_ids=list(range(NCORES)), trace=trace
    )
    lp = np.stack([np.asarray(r["out_lp"]).astype(np.float32)
                   for r in res.results])
    gp = np.stack([np.asarray(r["out_gp"]).astype(np.float32)
                   for r in res.results])
    return (lp, gp), res


def kernel(**inputs):
    out, _ = _run(inputs, trace=False)
    return out
